# revision 9
# baseline (speedup 1.0000x reference)
"""Trainium2 Bass kernel for AttentionalPlanarRemapping.

out[n,c,h,w] = sum_d softmax(atts[n,c,:])[d] * images[n,d,h,w]

Per-sample: W = softmax(atts[n]) [C,C]; out[n] = W @ images[n].reshape(C, H*W).

Sharding: data-parallel over N across 8 cores (4 samples per core).

Host preprocessing inside kernel(): atts is passed TRANSPOSED per sample
(attsT[n] = atts[n].T, layout [d, c]) and converted to fp16, so attsT loads
with the contraction dim d on partitions (the matmul lhsT layout) at half
the DMA cost. images are uploaded fp16 and the output stored fp16: the
rel-err budget (2e-2) dwarfs fp16 rounding, and HBM bandwidth is the
co-bottleneck with the PE.

Per-core plan (software-pipelined one sample ahead):
  prep(n):  per kd-block (128 rows of the contraction dim):
    DMA attsT chunk -> A[:, kd];  DMA images chunk -> X[:, kd];
    E[:, kd] = exp(A[:, kd])  (ACT, fp16; no max-sub: |atts| < 6)
    then T = sum_kd E  (3 DVE adds, fp16)  -- the free-axis half of the
    softmax denominator.
  compute(n):
    per kc (128 output channels):
      psum[c128, hw] += E[:, kd, kc-blk].T @ X[:, kd, ht-blk]  (8 matmuls)
      after kc=0: 4 tiny matmuls T_blk.T @ ones[128,2] finish the
      denominator (partition-sum) directly in per-partition layout;
      r = 1/s (DVE)
      evict psum -> O fp16 scaled by r[kc] (alternating ACT/DVE), then
      DMA O -> out[n] (alternating SWDGE/ACT-HWDGE queues so stores do
      not block the sync load queue)

A short stream of dummy matmuls at t=0 keeps the PE busy so the HAM clock
gate lifts (1.2 -> 2.4 GHz) before the real matmuls arrive.
"""

import numpy as np
from contextlib import ExitStack

import concourse.bass as bass
import concourse.mybir as mybir
import concourse.tile as tile
from concourse import bacc
from concourse.bass_utils import run_bass_kernel_spmd

N, C, H, W = 32, 512, 32, 32
HW = H * W                      # 1024
NCORES = 8
NPC = N // NCORES               # 4 samples per core
P = 128
KC = C // P                     # 4 chunks over output channel c
KD = C // P                     # 4 chunks over contraction d
NT = 512                        # matmul moving free dim (one PSUM bank of f32)
NHT = HW // NT                  # 2
NWARM = 20                      # dummy matmuls to lift the HAM clock gate

F32 = mybir.dt.float32
F16 = mybir.dt.float16
AF = mybir.ActivationFunctionType
OP = mybir.AluOpType


def build_nc():
    nc = bacc.Bacc("TRN2", target_bir_lowering=False, debug=False)

    images = nc.dram_tensor("images", [NPC, C, HW], F16, kind="ExternalInput").ap()
    attsT = nc.dram_tensor("attsT", [NPC, C, C], F16, kind="ExternalInput").ap()
    out = nc.dram_tensor("out", [NPC, C, HW], F16, kind="ExternalOutput").ap()

    with ExitStack() as ctx:
        tc = ctx.enter_context(tile.TileContext(nc))

        const_pool = ctx.enter_context(tc.tile_pool(name="const", bufs=1))
        ones_f32 = const_pool.tile([P, P], F32)
        ones = const_pool.tile([P, P], F16)
        ones2_f32 = const_pool.tile([P, 2], F32)
        ones2 = const_pool.tile([P, 2], F16)

        a_pool = ctx.enter_context(tc.tile_pool(name="a", bufs=2))
        e_pool = ctx.enter_context(tc.tile_pool(name="e", bufs=2))
        t_pool = ctx.enter_context(tc.tile_pool(name="t", bufs=2))
        x_pool = ctx.enter_context(tc.tile_pool(name="x", bufs=3))
        o_pool = ctx.enter_context(tc.tile_pool(name="o", bufs=2))
        r_pool = ctx.enter_context(tc.tile_pool(name="r", bufs=2))
        sm_psum = ctx.enter_context(tc.tile_pool(name="smp", bufs=1, space="PSUM"))
        mm_psum = ctx.enter_context(tc.tile_pool(name="mmp", bufs=3, space="PSUM"))

        def consts_and_warmup():
            nc.vector.memset(ones_f32[:], 1.0)
            nc.vector.tensor_copy(ones[:], ones_f32[:])
            nc.vector.memset(ones2_f32[:], 1.0)
            nc.vector.tensor_copy(ones2[:], ones2_f32[:])
            dummy_ps = sm_psum.tile([P, P], F32, tag="warm", space="PSUM")
            for _ in range(NWARM):
                nc.tensor.matmul(dummy_ps[:], lhsT=ones[:], rhs=ones[:])

        def prep(n):
            """Input DMAs + exp + kd-sum for sample n.

            One big DMA per tensor: the sync sequencer's DIRECT2D issue cost
            (~0.6us) is per-dma_start, so small chunked loads serialize the
            whole load stream on issue. Sample 0's images load is split in
            two so its first matmuls are not gated on the full 1MB.
            """
            a_t = a_pool.tile([P, KD, C], F16, name=f"a{n}", tag="a")
            x_t = x_pool.tile([P, KD, HW], F16, name=f"x{n}", tag="x")
            if n == 0:
                # startup is load-bound: interleave small per-kd chunks so
                # exp/matmuls start as soon as the first 128 contraction
                # rows have landed
                for kd in range(KD):
                    nc.sync.dma_start(
                        a_t[:, kd], attsT[n][kd * P : (kd + 1) * P]
                    )
                    nc.sync.dma_start(
                        x_t[:, kd], images[n][kd * P : (kd + 1) * P]
                    )
            else:
                nc.sync.dma_start(
                    a_t[:], attsT[n].rearrange("(kd p) c -> p kd c", p=P)
                )
                nc.sync.dma_start(
                    x_t[:], images[n].rearrange("(kd p) f -> p kd f", p=P)
                )
            e_t = e_pool.tile([P, KD, C], F16, name=f"e{n}", tag="e")
            for kd in range(KD):
                nc.scalar.activation(
                    e_t[:, kd], a_t[:, kd], AF.Exp, bias=0.0, scale=1.0
                )
            # T[d_p, c] = sum_kd E[d_p, kd, c]: free-axis half of the
            # denominator; the partition half happens in tiny matmuls below
            t2 = t_pool.tile([P, 2, C], F16, name=f"t2_{n}", tag="t2")
            nc.vector.scalar_tensor_tensor(
                t2[:, 0], e_t[:, 0], 1.0, e_t[:, 1], op0=OP.mult, op1=OP.add
            )
            nc.vector.scalar_tensor_tensor(
                t2[:, 1], e_t[:, 2], 1.0, e_t[:, 3], op0=OP.mult, op1=OP.add
            )
            tsum = t_pool.tile([P, C], F16, name=f"ts{n}", tag="ts")
            nc.vector.scalar_tensor_tensor(
                tsum[:], t2[:, 0], 1.0, t2[:, 1], op0=OP.mult, op1=OP.add
            )
            return e_t, x_t, tsum

        def compute(n, e_t, x_t, tsum):
            r_sb = None
            for kc in range(KC):
                ps = mm_psum.tile(
                    [P, HW], F32, name=f"ps{n}_{kc}", tag="ps", space="PSUM"
                )
                last_band = n == NPC - 1 and kc == KC - 1
                if last_band:
                    # tail: ht-major so the first half evicts/stores while
                    # the second half is still accumulating
                    o_t = o_pool.tile(
                        [P, HW], F16, name=f"o{n}_{kc}", tag=f"o{kc}"
                    )
                    for ht in range(NHT):
                        for kd in range(KD):
                            nc.tensor.matmul(
                                ps[:, ht * NT : (ht + 1) * NT],
                                lhsT=e_t[:, kd, kc * P : (kc + 1) * P],
                                rhs=x_t[:, kd, ht * NT : (ht + 1) * NT],
                                start=(kd == 0),
                                stop=(kd == KD - 1),
                            )
                        sl = slice(ht * NT, (ht + 1) * NT)
                        nc.scalar.mul(o_t[:, sl], ps[:, sl], r_sb[:, kc : kc + 1])
                        eng = nc.gpsimd if ht == 0 else nc.scalar
                        eng.dma_start(
                            out[n][kc * P : (kc + 1) * P, sl], o_t[:, sl]
                        )
                    continue
                for kd in range(KD):
                    for ht in range(NHT):
                        nc.tensor.matmul(
                            ps[:, ht * NT : (ht + 1) * NT],
                            lhsT=e_t[:, kd, kc * P : (kc + 1) * P],
                            rhs=x_t[:, kd, ht * NT : (ht + 1) * NT],
                            start=(kd == 0),
                            stop=(kd == KD - 1),
                        )
                if kc == 0:
                    # s[c] = sum_p T[p, c] via tiny matmuls: lands the
                    # denominator directly on the output-channel partitions
                    rp_ps = sm_psum.tile(
                        [P, 2 * KC], F32, name=f"rp{n}", tag="rp", space="PSUM"
                    )
                    for j in range(KC):
                        nc.tensor.matmul(
                            rp_ps[:, j * 2 : (j + 1) * 2],
                            lhsT=tsum[:, j * P : (j + 1) * P],
                            rhs=ones2[:],
                        )
                    s_col = r_pool.tile([P, KC], F32, name=f"scol{n}", tag="scol")
                    nc.vector.tensor_copy(
                        s_col[:],
                        rp_ps[:].rearrange("p (kc j) -> p kc j", j=2)[:, :, 0],
                    )
                    r_sb = r_pool.tile([P, KC], F32, name=f"rsb{n}", tag="rsb")
                    nc.vector.reciprocal(r_sb[:], s_col[:])
                # per-kc eviction + store: normalize while copying psum->SBUF
                o_t = o_pool.tile([P, HW], F16, name=f"o{n}_{kc}", tag=f"o{kc}")
                r_ap = r_sb[:, kc : kc + 1]
                if kc % 2 == 0:
                    nc.scalar.mul(o_t[:], ps[:], r_ap)
                    nc.gpsimd.dma_start(out[n][kc * P : (kc + 1) * P], o_t[:])
                else:
                    nc.vector.tensor_scalar_mul(o_t[:], ps[:], r_ap)
                    nc.scalar.dma_start(out[n][kc * P : (kc + 1) * P], o_t[:])

        consts_and_warmup()
        # software pipeline: prep one sample ahead so the next sample's
        # exp/loads are never queued behind this sample's evictions
        staged = prep(0)
        for n in range(NPC):
            nxt = prep(n + 1) if n + 1 < NPC else None
            compute(n, *staged)
            staged = nxt

    nc.compile()
    return nc


_NC_CACHE = None


def _get_nc():
    global _NC_CACHE
    if _NC_CACHE is None:
        _NC_CACHE = build_nc()
    return _NC_CACHE


def run(in_maps, **kwargs):
    """Run the SPMD kernel on cores 0..7. in_maps: one dict per core."""
    nc = _get_nc()
    return run_bass_kernel_spmd(nc, in_maps, core_ids=list(range(NCORES)), **kwargs)


def make_in_maps(images: np.ndarray, atts: np.ndarray):
    images = np.asarray(images, dtype=np.float32).astype(np.float16)
    atts = np.asarray(atts, dtype=np.float32)
    assert images.shape == (N, C, H, W), images.shape
    assert atts.shape == (N, C, C), atts.shape
    img_s = images.reshape(NCORES, NPC, C, HW)
    # per-sample transpose: attsT[n] = atts[n].T  (layout [d, c])
    attsT = np.ascontiguousarray(atts.transpose(0, 2, 1)).astype(np.float16)
    attsT = attsT.reshape(NCORES, NPC, C, C)
    return [
        {"images": np.ascontiguousarray(img_s[i]), "attsT": attsT[i]}
        for i in range(NCORES)
    ]


def kernel(images: np.ndarray, atts: np.ndarray) -> np.ndarray:
    in_maps = make_in_maps(images, atts)
    res = run(in_maps)
    outs = [res.results[i]["out"] for i in range(NCORES)]
    full = np.concatenate(outs, axis=0).reshape(N, C, H, W)
    return full.astype(np.float32)


# revision 11
# speedup vs baseline: 1.0704x; 1.0704x over previous
"""Trainium2 Bass kernel for AttentionalPlanarRemapping.

out[n,c,h,w] = sum_d softmax(atts[n,c,:])[d] * images[n,d,h,w]

Per-sample: W = softmax(atts[n]) [C,C]; out[n] = W @ images[n].reshape(C, H*W).

Sharding: data-parallel over N across 8 cores (4 samples per core).

Host preprocessing inside kernel(): atts is passed TRANSPOSED per sample
(attsT[n] = atts[n].T, layout [d, c]) and converted to fp16, so attsT loads
with the contraction dim d on partitions (the matmul lhsT layout) at half
the DMA cost. images are uploaded fp16 and the output stored fp16: the
rel-err budget (2e-2) dwarfs fp16 rounding, and HBM bandwidth is the
co-bottleneck with the PE.

Per-core plan (software-pipelined one sample ahead):
  prep(n):  per kd-block (128 rows of the contraction dim):
    DMA attsT chunk -> A[:, kd];  DMA images chunk -> X[:, kd];
    E[:, kd] = exp(A[:, kd])  (ACT, fp16; no max-sub: |atts| < 6)
    then T = sum_kd E  (3 DVE adds, fp16)  -- the free-axis half of the
    softmax denominator.
  compute(n):
    per kc (128 output channels):
      psum[c128, hw] += E[:, kd, kc-blk].T @ X[:, kd, ht-blk]  (8 matmuls)
      after kc=0: 4 tiny matmuls T_blk.T @ ones[128,2] finish the
      denominator (partition-sum) directly in per-partition layout;
      r = 1/s (DVE)
      evict psum -> O fp16 scaled by r[kc] (alternating ACT/DVE), then
      DMA O -> out[n] (alternating SWDGE/ACT-HWDGE queues so stores do
      not block the sync load queue)

A short stream of dummy matmuls at t=0 keeps the PE busy so the HAM clock
gate lifts (1.2 -> 2.4 GHz) before the real matmuls arrive.
"""

import numpy as np
from contextlib import ExitStack

import concourse.bass as bass
import concourse.mybir as mybir
import concourse.tile as tile
from concourse import bacc
from concourse.bass_utils import run_bass_kernel_spmd

N, C, H, W = 32, 512, 32, 32
HW = H * W                      # 1024
NCORES = 8
NPC = N // NCORES               # 4 samples per core
P = 128
KC = C // P                     # 4 chunks over output channel c
KD = C // P                     # 4 chunks over contraction d
NT = 512                        # matmul moving free dim (one PSUM bank of f32)
NHT = HW // NT                  # 2
NWARM = 45                      # dummy matmuls to lift the HAM clock gate

F32 = mybir.dt.float32
F16 = mybir.dt.float16
AF = mybir.ActivationFunctionType
OP = mybir.AluOpType


def build_nc():
    nc = bacc.Bacc("TRN2", target_bir_lowering=False, debug=False)

    images = nc.dram_tensor("images", [NPC, C, HW], F16, kind="ExternalInput").ap()
    attsT = nc.dram_tensor("attsT", [NPC, C, C], F16, kind="ExternalInput").ap()
    out = nc.dram_tensor("out", [NPC, C, HW], F16, kind="ExternalOutput").ap()

    with ExitStack() as ctx:
        tc = ctx.enter_context(tile.TileContext(nc))

        const_pool = ctx.enter_context(tc.tile_pool(name="const", bufs=1))
        ones_f32 = const_pool.tile([P, P], F32)
        ones = const_pool.tile([P, P], F16)
        ones2_f32 = const_pool.tile([P, 2], F32)
        ones2 = const_pool.tile([P, 2], F16)

        a_pool = ctx.enter_context(tc.tile_pool(name="a", bufs=2))
        e_pool = ctx.enter_context(tc.tile_pool(name="e", bufs=2))
        t_pool = ctx.enter_context(tc.tile_pool(name="t", bufs=2))
        x_pool = ctx.enter_context(tc.tile_pool(name="x", bufs=3))
        o_pool = ctx.enter_context(tc.tile_pool(name="o", bufs=2))
        r_pool = ctx.enter_context(tc.tile_pool(name="r", bufs=2))
        sm_psum = ctx.enter_context(tc.tile_pool(name="smp", bufs=1, space="PSUM"))
        mm_psum = ctx.enter_context(tc.tile_pool(name="mmp", bufs=3, space="PSUM"))

        def consts_and_warmup():
            nc.vector.memset(ones_f32[:], 1.0)
            nc.vector.tensor_copy(ones[:], ones_f32[:])
            nc.vector.memset(ones2_f32[:], 1.0)
            nc.vector.tensor_copy(ones2[:], ones2_f32[:])
            dummy_ps = sm_psum.tile([P, P], F32, tag="warm", space="PSUM")
            for _ in range(NWARM):
                nc.tensor.matmul(dummy_ps[:], lhsT=ones[:], rhs=ones[:])

        def prep(n):
            """Input DMAs + exp + kd-sum for sample n.

            One big DMA per tensor: the sync sequencer's DIRECT2D issue cost
            (~0.6us) is per-dma_start, so small chunked loads serialize the
            whole load stream on issue. Sample 0's images load is split in
            two so its first matmuls are not gated on the full 1MB.
            """
            a_t = a_pool.tile([P, KD, C], F16, name=f"a{n}", tag="a")
            x_t = x_pool.tile([P, KD, HW], F16, name=f"x{n}", tag="x")
            nc.sync.dma_start(
                a_t[:], attsT[n].rearrange("(kd p) c -> p kd c", p=P)
            )
            if n == 0:
                for h in range(2):
                    nc.sync.dma_start(
                        x_t[:, h * 2 : (h + 1) * 2],
                        images[n][h * 256 : (h + 1) * 256].rearrange(
                            "(kd p) f -> p kd f", p=P
                        ),
                    )
            else:
                nc.sync.dma_start(
                    x_t[:], images[n].rearrange("(kd p) f -> p kd f", p=P)
                )
            e_t = e_pool.tile([P, KD, C], F16, name=f"e{n}", tag="e")
            for kd in range(KD):
                nc.scalar.activation(
                    e_t[:, kd], a_t[:, kd], AF.Exp, bias=0.0, scale=1.0
                )
            # T[d_p, c] = sum_kd E[d_p, kd, c]: free-axis half of the
            # denominator; the partition half happens in tiny matmuls below
            t2 = t_pool.tile([P, 2, C], F16, name=f"t2_{n}", tag="t2")
            nc.vector.scalar_tensor_tensor(
                t2[:, 0], e_t[:, 0], 1.0, e_t[:, 1], op0=OP.mult, op1=OP.add
            )
            nc.vector.scalar_tensor_tensor(
                t2[:, 1], e_t[:, 2], 1.0, e_t[:, 3], op0=OP.mult, op1=OP.add
            )
            tsum = t_pool.tile([P, C], F16, name=f"ts{n}", tag="ts")
            nc.vector.scalar_tensor_tensor(
                tsum[:], t2[:, 0], 1.0, t2[:, 1], op0=OP.mult, op1=OP.add
            )
            return e_t, x_t, tsum

        def compute(n, e_t, x_t, tsum):
            r_sb = None
            for kc in range(KC):
                ps = mm_psum.tile(
                    [P, HW], F32, name=f"ps{n}_{kc}", tag="ps", space="PSUM"
                )
                last_band = n == NPC - 1 and kc == KC - 1
                if last_band:
                    # tail: ht-major so the first half evicts/stores while
                    # the second half is still accumulating
                    o_t = o_pool.tile(
                        [P, HW], F16, name=f"o{n}_{kc}", tag=f"o{kc}"
                    )
                    for ht in range(NHT):
                        for kd in range(KD):
                            nc.tensor.matmul(
                                ps[:, ht * NT : (ht + 1) * NT],
                                lhsT=e_t[:, kd, kc * P : (kc + 1) * P],
                                rhs=x_t[:, kd, ht * NT : (ht + 1) * NT],
                                start=(kd == 0),
                                stop=(kd == KD - 1),
                            )
                        sl = slice(ht * NT, (ht + 1) * NT)
                        nc.scalar.mul(o_t[:, sl], ps[:, sl], r_sb[:, kc : kc + 1])
                        eng = nc.gpsimd if ht == 0 else nc.scalar
                        eng.dma_start(
                            out[n][kc * P : (kc + 1) * P, sl], o_t[:, sl]
                        )
                    continue
                for kd in range(KD):
                    for ht in range(NHT):
                        nc.tensor.matmul(
                            ps[:, ht * NT : (ht + 1) * NT],
                            lhsT=e_t[:, kd, kc * P : (kc + 1) * P],
                            rhs=x_t[:, kd, ht * NT : (ht + 1) * NT],
                            start=(kd == 0),
                            stop=(kd == KD - 1),
                        )
                if kc == 0:
                    # s[c] = sum_p T[p, c] via tiny matmuls: lands the
                    # denominator directly on the output-channel partitions
                    rp_ps = sm_psum.tile(
                        [P, 2 * KC], F32, name=f"rp{n}", tag="rp", space="PSUM"
                    )
                    for j in range(KC):
                        nc.tensor.matmul(
                            rp_ps[:, j * 2 : (j + 1) * 2],
                            lhsT=tsum[:, j * P : (j + 1) * P],
                            rhs=ones2[:],
                        )
                    s_col = r_pool.tile([P, KC], F32, name=f"scol{n}", tag="scol")
                    nc.vector.tensor_copy(
                        s_col[:],
                        rp_ps[:].rearrange("p (kc j) -> p kc j", j=2)[:, :, 0],
                    )
                    r_sb = r_pool.tile([P, KC], F32, name=f"rsb{n}", tag="rsb")
                    nc.vector.reciprocal(r_sb[:], s_col[:])
                # per-kc eviction + store: normalize while copying psum->SBUF
                o_t = o_pool.tile([P, HW], F16, name=f"o{n}_{kc}", tag=f"o{kc}")
                r_ap = r_sb[:, kc : kc + 1]
                if kc % 2 == 0:
                    nc.scalar.mul(o_t[:], ps[:], r_ap)
                    nc.gpsimd.dma_start(out[n][kc * P : (kc + 1) * P], o_t[:])
                else:
                    nc.vector.tensor_scalar_mul(o_t[:], ps[:], r_ap)
                    nc.scalar.dma_start(out[n][kc * P : (kc + 1) * P], o_t[:])

        consts_and_warmup()
        # software pipeline: prep one sample ahead so the next sample's
        # exp/loads are never queued behind this sample's evictions
        staged = prep(0)
        for n in range(NPC):
            nxt = prep(n + 1) if n + 1 < NPC else None
            compute(n, *staged)
            staged = nxt

    nc.compile()
    return nc


_NC_CACHE = None


def _get_nc():
    global _NC_CACHE
    if _NC_CACHE is None:
        _NC_CACHE = build_nc()
    return _NC_CACHE


def run(in_maps, **kwargs):
    """Run the SPMD kernel on cores 0..7. in_maps: one dict per core."""
    nc = _get_nc()
    return run_bass_kernel_spmd(nc, in_maps, core_ids=list(range(NCORES)), **kwargs)


def make_in_maps(images: np.ndarray, atts: np.ndarray):
    images = np.asarray(images, dtype=np.float32).astype(np.float16)
    atts = np.asarray(atts, dtype=np.float32)
    assert images.shape == (N, C, H, W), images.shape
    assert atts.shape == (N, C, C), atts.shape
    img_s = images.reshape(NCORES, NPC, C, HW)
    # per-sample transpose: attsT[n] = atts[n].T  (layout [d, c])
    attsT = np.ascontiguousarray(atts.transpose(0, 2, 1)).astype(np.float16)
    attsT = attsT.reshape(NCORES, NPC, C, C)
    return [
        {"images": np.ascontiguousarray(img_s[i]), "attsT": attsT[i]}
        for i in range(NCORES)
    ]


def kernel(images: np.ndarray, atts: np.ndarray) -> np.ndarray:
    in_maps = make_in_maps(images, atts)
    res = run(in_maps)
    outs = [res.results[i]["out"] for i in range(NCORES)]
    full = np.concatenate(outs, axis=0).reshape(N, C, H, W)
    return full.astype(np.float32)


# revision 12
# speedup vs baseline: 1.1430x; 1.0679x over previous
"""Trainium2 Bass kernel for AttentionalPlanarRemapping.

out[n,c,h,w] = sum_d softmax(atts[n,c,:])[d] * images[n,d,h,w]

Per-sample: W = softmax(atts[n]) [C,C]; out[n] = W @ images[n].reshape(C, H*W).

Sharding: data-parallel over N across 8 cores (4 samples per core).

Host preprocessing inside kernel(): atts is passed TRANSPOSED per sample
(attsT[n] = atts[n].T, layout [d, c]) and converted to fp16, so attsT loads
with the contraction dim d on partitions (the matmul lhsT layout) at half
the DMA cost. images are uploaded fp16 and the output stored fp16: the
rel-err budget (2e-2) dwarfs fp16 rounding, and HBM bandwidth is the
co-bottleneck with the PE.

Per-core plan (software-pipelined one sample ahead):
  prep(n):  per kd-block (128 rows of the contraction dim):
    DMA attsT chunk -> A[:, kd];  DMA images chunk -> X[:, kd];
    E[:, kd] = exp(A[:, kd])  (ACT, fp16; no max-sub: |atts| < 6)
    then T = sum_kd E  (3 DVE adds, fp16)  -- the free-axis half of the
    softmax denominator.
  compute(n):
    per kc (128 output channels):
      psum[c128, hw] += E[:, kd, kc-blk].T @ X[:, kd, ht-blk]  (8 matmuls)
      after kc=0: 4 tiny matmuls T_blk.T @ ones[128,2] finish the
      denominator (partition-sum) directly in per-partition layout;
      r = 1/s (DVE)
      evict psum -> O fp16 scaled by r[kc] (alternating ACT/DVE), then
      DMA O -> out[n] (alternating SWDGE/ACT-HWDGE queues so stores do
      not block the sync load queue)

A short stream of dummy matmuls at t=0 keeps the PE busy so the HAM clock
gate lifts (1.2 -> 2.4 GHz) before the real matmuls arrive.
"""

import numpy as np
from contextlib import ExitStack

import concourse.bass as bass
import concourse.mybir as mybir
import concourse.tile as tile
from concourse import bacc
from concourse.bass_utils import run_bass_kernel_spmd

N, C, H, W = 32, 512, 32, 32
HW = H * W                      # 1024
NCORES = 8
NPC = N // NCORES               # 4 samples per core
P = 128
KC = C // P                     # 4 chunks over output channel c
KD = C // P                     # 4 chunks over contraction d
NT = 512                        # matmul moving free dim (one PSUM bank of f32)
NHT = HW // NT                  # 2
NWARM = 20                      # dummy matmuls to lift the HAM clock gate
                                # (must drain before the first load's DMA
                                # completes — a busy PE delays the completion
                                # semaphore 1:1, starving the exp chain)

F32 = mybir.dt.float32
F16 = mybir.dt.float16
AF = mybir.ActivationFunctionType
OP = mybir.AluOpType


def build_nc():
    nc = bacc.Bacc("TRN2", target_bir_lowering=False, debug=False)

    images = nc.dram_tensor("images", [NPC, C, HW], F16, kind="ExternalInput").ap()
    attsT = nc.dram_tensor("attsT", [NPC, C, C], F16, kind="ExternalInput").ap()
    out = nc.dram_tensor("out", [NPC, C, HW], F16, kind="ExternalOutput").ap()

    with ExitStack() as ctx:
        tc = ctx.enter_context(tile.TileContext(nc))

        const_pool = ctx.enter_context(tc.tile_pool(name="const", bufs=1))
        ones_f32 = const_pool.tile([P, P], F32)
        ones = const_pool.tile([P, P], F16)
        ones2_f32 = const_pool.tile([P, 2], F32)
        ones2 = const_pool.tile([P, 2], F16)

        a_pool = ctx.enter_context(tc.tile_pool(name="a", bufs=2))
        e_pool = ctx.enter_context(tc.tile_pool(name="e", bufs=2))
        t_pool = ctx.enter_context(tc.tile_pool(name="t", bufs=2))
        x_pool = ctx.enter_context(tc.tile_pool(name="x", bufs=3))
        o_pool = ctx.enter_context(tc.tile_pool(name="o", bufs=2))
        r_pool = ctx.enter_context(tc.tile_pool(name="r", bufs=2))
        sm_psum = ctx.enter_context(tc.tile_pool(name="smp", bufs=1, space="PSUM"))
        mm_psum = ctx.enter_context(tc.tile_pool(name="mmp", bufs=3, space="PSUM"))

        def consts_and_warmup():
            nc.vector.memset(ones_f32[:], 1.0)
            nc.vector.tensor_copy(ones[:], ones_f32[:])
            nc.vector.memset(ones2_f32[:], 1.0)
            nc.vector.tensor_copy(ones2[:], ones2_f32[:])
            dummy_ps = sm_psum.tile([P, P], F32, tag="warm", space="PSUM")
            for _ in range(NWARM):
                nc.tensor.matmul(dummy_ps[:], lhsT=ones[:], rhs=ones[:])

        def prep(n):
            """Input DMAs + exp + kd-sum for sample n.

            One big DMA per tensor: the sync sequencer's DIRECT2D issue cost
            (~0.6us) is per-dma_start, so small chunked loads serialize the
            whole load stream on issue. Sample 0's images load is split in
            two so its first matmuls are not gated on the full 1MB.
            """
            a_t = a_pool.tile([P, KD, C], F16, name=f"a{n}", tag="a")
            x_t = x_pool.tile([P, KD, HW], F16, name=f"x{n}", tag="x")
            nc.sync.dma_start(
                a_t[:], attsT[n].rearrange("(kd p) c -> p kd c", p=P)
            )
            if n == 0:
                for h in range(2):
                    nc.sync.dma_start(
                        x_t[:, h * 2 : (h + 1) * 2],
                        images[n][h * 256 : (h + 1) * 256].rearrange(
                            "(kd p) f -> p kd f", p=P
                        ),
                    )
            else:
                nc.sync.dma_start(
                    x_t[:], images[n].rearrange("(kd p) f -> p kd f", p=P)
                )
            e_t = e_pool.tile([P, KD, C], F16, name=f"e{n}", tag="e")
            for kd in range(KD):
                nc.scalar.activation(
                    e_t[:, kd], a_t[:, kd], AF.Exp, bias=0.0, scale=1.0
                )
            # T[d_p, c] = sum_kd E[d_p, kd, c]: free-axis half of the
            # denominator; the partition half happens in tiny matmuls below
            t2 = t_pool.tile([P, 2, C], F16, name=f"t2_{n}", tag="t2")
            nc.vector.scalar_tensor_tensor(
                t2[:, 0], e_t[:, 0], 1.0, e_t[:, 1], op0=OP.mult, op1=OP.add
            )
            nc.vector.scalar_tensor_tensor(
                t2[:, 1], e_t[:, 2], 1.0, e_t[:, 3], op0=OP.mult, op1=OP.add
            )
            tsum = t_pool.tile([P, C], F16, name=f"ts{n}", tag="ts")
            nc.vector.scalar_tensor_tensor(
                tsum[:], t2[:, 0], 1.0, t2[:, 1], op0=OP.mult, op1=OP.add
            )
            return e_t, x_t, tsum

        def compute(n, e_t, x_t, tsum):
            r_sb = None
            for kc in range(KC):
                ps = mm_psum.tile(
                    [P, HW], F32, name=f"ps{n}_{kc}", tag="ps", space="PSUM"
                )
                last_band = n == NPC - 1 and kc == KC - 1
                if last_band:
                    # tail: ht-major so the first half evicts/stores while
                    # the second half is still accumulating
                    o_t = o_pool.tile(
                        [P, HW], F16, name=f"o{n}_{kc}", tag=f"o{kc}"
                    )
                    for ht in range(NHT):
                        for kd in range(KD):
                            nc.tensor.matmul(
                                ps[:, ht * NT : (ht + 1) * NT],
                                lhsT=e_t[:, kd, kc * P : (kc + 1) * P],
                                rhs=x_t[:, kd, ht * NT : (ht + 1) * NT],
                                start=(kd == 0),
                                stop=(kd == KD - 1),
                            )
                        sl = slice(ht * NT, (ht + 1) * NT)
                        nc.scalar.mul(o_t[:, sl], ps[:, sl], r_sb[:, kc : kc + 1])
                        eng = nc.gpsimd if ht == 0 else nc.scalar
                        eng.dma_start(
                            out[n][kc * P : (kc + 1) * P, sl], o_t[:, sl]
                        )
                    continue
                for kd in range(KD):
                    for ht in range(NHT):
                        nc.tensor.matmul(
                            ps[:, ht * NT : (ht + 1) * NT],
                            lhsT=e_t[:, kd, kc * P : (kc + 1) * P],
                            rhs=x_t[:, kd, ht * NT : (ht + 1) * NT],
                            start=(kd == 0),
                            stop=(kd == KD - 1),
                        )
                if kc == 0:
                    # s[c] = sum_p T[p, c] via tiny matmuls: lands the
                    # denominator directly on the output-channel partitions
                    rp_ps = sm_psum.tile(
                        [P, 2 * KC], F32, name=f"rp{n}", tag="rp", space="PSUM"
                    )
                    for j in range(KC):
                        nc.tensor.matmul(
                            rp_ps[:, j * 2 : (j + 1) * 2],
                            lhsT=tsum[:, j * P : (j + 1) * P],
                            rhs=ones2[:],
                        )
                    s_col = r_pool.tile([P, KC], F32, name=f"scol{n}", tag="scol")
                    nc.vector.tensor_copy(
                        s_col[:],
                        rp_ps[:].rearrange("p (kc j) -> p kc j", j=2)[:, :, 0],
                    )
                    r_sb = r_pool.tile([P, KC], F32, name=f"rsb{n}", tag="rsb")
                    nc.vector.reciprocal(r_sb[:], s_col[:])
                # per-kc eviction + store: normalize while copying psum->SBUF
                o_t = o_pool.tile([P, HW], F16, name=f"o{n}_{kc}", tag=f"o{kc}")
                r_ap = r_sb[:, kc : kc + 1]
                if kc % 2 == 0:
                    nc.scalar.mul(o_t[:], ps[:], r_ap)
                    nc.gpsimd.dma_start(out[n][kc * P : (kc + 1) * P], o_t[:])
                else:
                    nc.vector.tensor_scalar_mul(o_t[:], ps[:], r_ap)
                    nc.scalar.dma_start(out[n][kc * P : (kc + 1) * P], o_t[:])

        consts_and_warmup()
        # software pipeline: prep one sample ahead so the next sample's
        # exp/loads are never queued behind this sample's evictions
        staged = prep(0)
        for n in range(NPC):
            nxt = prep(n + 1) if n + 1 < NPC else None
            compute(n, *staged)
            staged = nxt

    nc.compile()
    return nc


_NC_CACHE = None


def _get_nc():
    global _NC_CACHE
    if _NC_CACHE is None:
        _NC_CACHE = build_nc()
    return _NC_CACHE


def run(in_maps, **kwargs):
    """Run the SPMD kernel on cores 0..7. in_maps: one dict per core."""
    nc = _get_nc()
    return run_bass_kernel_spmd(nc, in_maps, core_ids=list(range(NCORES)), **kwargs)


def make_in_maps(images: np.ndarray, atts: np.ndarray):
    images = np.asarray(images, dtype=np.float32).astype(np.float16)
    atts = np.asarray(atts, dtype=np.float32)
    assert images.shape == (N, C, H, W), images.shape
    assert atts.shape == (N, C, C), atts.shape
    img_s = images.reshape(NCORES, NPC, C, HW)
    # per-sample transpose: attsT[n] = atts[n].T  (layout [d, c])
    attsT = np.ascontiguousarray(atts.transpose(0, 2, 1)).astype(np.float16)
    attsT = attsT.reshape(NCORES, NPC, C, C)
    return [
        {"images": np.ascontiguousarray(img_s[i]), "attsT": attsT[i]}
        for i in range(NCORES)
    ]


def kernel(images: np.ndarray, atts: np.ndarray) -> np.ndarray:
    in_maps = make_in_maps(images, atts)
    res = run(in_maps)
    outs = [res.results[i]["out"] for i in range(NCORES)]
    full = np.concatenate(outs, axis=0).reshape(N, C, H, W)
    return full.astype(np.float32)
